# revision 27
# baseline (speedup 1.0000x reference)
"""GAT layer on 8 Trainium2 NeuronCores (Bass/Tile), edge-parallel dst-sharded.

v4: zero per-edge DMA gathering. The host knows every edge at build time, so
it pre-gathers x[src] into a contiguous per-edge-slot array; the device
computes per-edge [Wh|el] rows by dense matmul (tensor engine), adds er[dst]
via a host-provided transposed one-hot matmul against the locally-computed er
table, and scatters with the usual one-hot matmul chain. All DMA is big and
contiguous; GPSIMD is not used at all.

Per chunk of 2 dst buckets (~38 blocks of 128 edge slots):
  - load XG (x[src].T halves) + OTT (dst one-hot, transposed) slices
  - per sub-group of 6 blocks:
      per block: 2 chained matmuls -> psG[128, 264] = [Wh | el] (f32 PSUM)
                 1 matmul psE[:, blk] = OTT.T @ er_bucket  (er per edge)
                 ACT copy el slice -> contiguous SBUF
      batched:   z = el + psE ; leaky ; exp -> w  (DVE + ACT)
      per block: V = psG[:, :256] * w (DVE, fused PSUM read), V[:,256:] = w
  - per bucket: chained one-hot scatter matmuls in PSUM, normalize, out
"""
import sys

for _p in ("/opt/trn_rl_repo",):
    if _p not in sys.path:
        sys.path.insert(0, _p)

import numpy as np
import ml_dtypes

import concourse.bass as bass
import concourse.tile as tile
from concourse import mybir
from concourse.bass_utils import run_bass_kernel_spmd

BF16 = ml_dtypes.bfloat16

N = 50000
E = 800000
IN = 256
H = 8
C = 32
HC = H * C            # 256
NC = 8
NPC = N // NC         # 6250 nodes per core
BUCKET = 128
NBUCK = (NPC + BUCKET - 1) // BUCKET   # 49
XT_PAD = NBUCK * 128                   # 6272
PAY = HC + H          # 264: [Wh | el]
CHUNKB = 2            # dst buckets per phase-2 chunk
SG = 3                # blocks per score sub-group (PSUM psG tiles alive)
NEG = 0.2
EPS = 1e-16

# walrus in this container caps sync waits per instruction at 1; hoist excess
# onto same-engine NoOps.
_waitfix_ctr = [0]


def _split_excess_waits(nc, max_waits=1):
    n_fixed = 0
    for fn in nc.m.functions:
        for bb in fn.blocks:
            insts = bb.instructions
            out = []
            for ins in insts:
                si = ins.sync_info
                waits = list(si.on_wait) if si is not None and si.on_wait else []
                if len(waits) > max_waits:
                    keep = waits[-max_waits:]
                    extra = waits[:-max_waits]
                    for i in range(0, len(extra), max_waits):
                        grp = extra[i:i + max_waits]
                        _waitfix_ctr[0] += 1
                        nop = mybir.InstNoOp(
                            name=f"I-waitfix-{_waitfix_ctr[0]}", ins=[], outs=[])
                        nop.engine = ins.engine
                        nop.sync_info = mybir.SyncInfo(on_wait=grp, on_update=[])
                        nc.register_instruction(nop)
                        out.append(nop)
                    si.on_wait = keep
                    n_fixed += 1
                out.append(ins)
            if len(out) != len(insts):
                bb.instructions = out
    return n_fixed


class Plan:
    """Compiled-in slot layout, identical across cores (SPMD)."""

    def __init__(self, caps):
        self.caps = caps                      # caps[b] = blocks for bucket b
        self.nchunk = (NBUCK + CHUNKB - 1) // CHUNKB
        self.chunks = []
        blk = 0
        for ci in range(self.nchunk):
            buckets = list(range(ci * CHUNKB, min((ci + 1) * CHUNKB, NBUCK)))
            boff = []
            off = 0
            for b in buckets:
                boff.append(off)
                off += caps[b]
            self.chunks.append({
                "buckets": buckets,
                "boff": boff,      # block offset of bucket within chunk
                "nb": off,
                "blk0": blk,
            })
            blk += off
        self.nblk = blk
        self.maxnb = max(c["nb"] for c in self.chunks)


def _host_prep(x, edge_index, W, a_left, a_right):
    src = np.concatenate([np.asarray(edge_index[0], np.int64),
                          np.arange(N, dtype=np.int64)])
    dst = np.concatenate([np.asarray(edge_index[1], np.int64),
                          np.arange(N, dtype=np.int64)])

    # fold attention vectors through W:  [el|er] = x @ (W.T @ A)
    A = np.zeros((HC, 2 * H), np.float32)
    for h in range(H):
        A[h * C:(h + 1) * C, h] = a_left[h]
        A[h * C:(h + 1) * C, H + h] = a_right[h]
    B = (W.T.astype(np.float64) @ A.astype(np.float64)).astype(np.float32)
    wtbW = np.concatenate([W.T.astype(np.float32), B[:, :H]], axis=1).astype(BF16)
    wtbR = np.ascontiguousarray(B[:, H:]).astype(BF16)          # [256, 8]

    core = dst // NPC
    counts = np.zeros((NC, NBUCK), np.int64)
    per_core = []
    for c in range(NC):
        m = core == c
        s_c, d_c = src[m], dst[m]
        dl = d_c - c * NPC
        b_c = dl // BUCKET
        np.add.at(counts[c], b_c, 1)
        per_core.append((s_c, dl, b_c))
    caps = ((counts.max(axis=0) + 127) // 128).tolist()
    plan = Plan(caps)
    nblk = plan.nblk
    nslot = nblk * 128

    bstart = np.zeros(NBUCK, np.int64)    # start slot of each bucket
    pos = 0
    for b in range(NBUCK):
        bstart[b] = pos
        pos += caps[b] * 128

    xgT = np.zeros((NC, IN, nslot), BF16)
    OTT = np.zeros((NC, 128, nslot), BF16)
    OTH = np.zeros((NC, 128, nslot), BF16)
    xT = np.zeros((NC, IN, XT_PAD), BF16)
    xbf = x.astype(BF16)

    for c in range(NC):
        s_c, dl, b_c = per_core[c]
        order = np.lexsort((s_c, b_c))
        s_c, dl, b_c = s_c[order], dl[order], b_c[order]
        # slot per edge: bucket-major, running position within bucket
        run = np.zeros(len(s_c), np.int64)
        uniq, first_pos, cnts = np.unique(b_c, return_index=True,
                                          return_counts=True)
        for u, fp, ct in zip(uniq, first_pos, cnts):
            run[fp:fp + ct] = np.arange(ct)
        slots = bstart[b_c] + run

        srcs = np.zeros(nslot, np.int64)          # pad slots -> node 0
        dlocv = np.full(nslot, 200.0, np.float32)
        srcs[slots] = s_c
        dlocv[slots] = (dl - b_c * BUCKET).astype(np.float32)

        xgT[c] = xbf[srcs].T                       # [256, nslot]
        OTT[c] = (dlocv[None, :] ==
                  np.arange(128, dtype=np.float32)[:, None]).astype(BF16)
        # scatter one-hot, edge-partition orientation: [p, blk*128 + d]
        dl2 = dlocv.reshape(nblk, 128)             # [blk, p]
        OTH[c] = (dl2[:, :, None] ==
                  np.arange(128, dtype=np.float32)[None, None, :]
                  ).astype(BF16).transpose(1, 0, 2).reshape(128, nslot)
        xT[c, :, :NPC] = xbf[c * NPC:(c + 1) * NPC].T

    return plan, wtbW, wtbR, xgT, OTT, OTH, xT


def _build_program(plan):
    f32 = mybir.dt.float32
    bf16 = mybir.dt.bfloat16
    nblk = plan.nblk
    nslot = nblk * 128

    nc = bass.Bass(trn_type="TRN2", num_devices=NC)
    xg0_in = nc.declare_dram_parameter("xg0", [128, nslot], bf16, isOutput=False)
    xg1_in = nc.declare_dram_parameter("xg1", [128, nslot], bf16, isOutput=False)
    ott_in = nc.declare_dram_parameter("ott", [128, nslot], bf16, isOutput=False)
    xT_in = nc.declare_dram_parameter("xT", [IN, XT_PAD], bf16, isOutput=False)
    wtbW_in = nc.declare_dram_parameter("wtbW", [IN, PAY], bf16, isOutput=False)
    wtbR_in = nc.declare_dram_parameter("wtbR", [IN, H], bf16, isOutput=False)
    oth_in = nc.declare_dram_parameter("oth", [128, nslot], bf16, isOutput=False)
    out_ext = nc.declare_dram_parameter("out", [NPC, HC], f32, isOutput=True)

    with tile.TileContext(nc) as tc:
        with tc.tile_pool(name="cst", bufs=1) as cst, \
             tc.tile_pool(name="gp", bufs=2) as gp, \
             tc.tile_pool(name="wp", bufs=2) as wp, \
             tc.tile_pool(name="np_", bufs=3) as np_, \
             tc.tile_pool(name="psg", bufs=5, space="PSUM") as psgp, \
             tc.tile_pool(name="ps2", bufs=3, space="PSUM") as ps2p:
            # PSUM is 8 banks, pool tiles are bank-granular, bufs count is
            # per tag: psg(psG)x5 + ps2(acc)x3 = 8 (phase 1 reuses psG)

            # ---------------- constants + phase 1 (er table) ----------------
            wW = []
            wR = []
            for k in range(2):
                t = cst.tile([128, PAY], bf16, tag=f"wW{k}")
                nc.sync.dma_start(out=t[:], in_=wtbW_in[k * 128:(k + 1) * 128, :])
                wW.append(t)
                u = cst.tile([128, H], bf16, tag=f"wR{k}")
                nc.sync.dma_start(out=u[:], in_=wtbR_in[k * 128:(k + 1) * 128, :])
                wR.append(u)

            er_all = cst.tile([128, NBUCK * H], bf16)   # er rows per bucket
            with tc.tile_pool(name="p1x", bufs=1) as p1x:
                xts = []
                for k in range(2):
                    v = p1x.tile([128, XT_PAD], bf16, tag=f"xt{k}")
                    nc.sync.dma_start(out=v[:],
                                      in_=xT_in[k * 128:(k + 1) * 128, :])
                    xts.append(v)
                for tn in range(NBUCK):
                    ps = psgp.tile([128, PAY], f32, tag="psG")
                    for k in range(2):
                        nc.tensor.matmul(
                            out=ps[:, 0:H],
                            lhsT=xts[k][:, tn * 128:(tn + 1) * 128],
                            rhs=wR[k][:],
                            start=(k == 0), stop=(k == 1),
                        )
                    nc.vector.tensor_copy(out=er_all[:, tn * H:(tn + 1) * H],
                                          in_=ps[:, 0:H])

            # ---------------- phase 2 ----------------
            for ch in plan.chunks:
                nb = ch["nb"]
                blk0 = ch["blk0"]
                s0 = blk0 * 128
                XG0 = gp.tile([128, nb * 128], bf16, tag="XG0")
                nc.sync.dma_start(out=XG0[:], in_=xg0_in[:, s0:s0 + nb * 128])
                XG1 = gp.tile([128, nb * 128], bf16, tag="XG1")
                nc.sync.dma_start(out=XG1[:], in_=xg1_in[:, s0:s0 + nb * 128])
                OTT = gp.tile([128, nb * 128], bf16, tag="OTT")
                nc.sync.dma_start(out=OTT[:], in_=ott_in[:, s0:s0 + nb * 128])
                OT = gp.tile([128, nb * 128], bf16, tag="OT")
                nc.sync.dma_start(out=OT[:], in_=oth_in[:, s0:s0 + nb * 128])

                # bucket of each block within this chunk
                blk_bucket = []
                for bi, b in enumerate(ch["buckets"]):
                    blk_bucket += [b] * plan.caps[b]

                Gsb = wp.tile([128, nb, PAY], bf16, tag="G")
                zt = wp.tile([128, nb * H], f32, tag="z")
                w_t = wp.tile([128, nb, H], bf16, tag="w")
                V = wp.tile([128, nb, PAY], bf16, tag="V")

                for blk in range(nb):
                    psG = psgp.tile([128, PAY], f32, tag="psG")
                    for k, XG in enumerate((XG0, XG1)):
                        nc.tensor.matmul(
                            out=psG[:],
                            lhsT=XG[:, blk * 128:(blk + 1) * 128],
                            rhs=wW[k][:],
                            start=(k == 0), stop=False,
                        )
                    # er accumulates straight onto el: psG[:,256:264] = el+er
                    b = blk_bucket[blk]
                    nc.tensor.matmul(
                        out=psG[:, HC:PAY],
                        lhsT=OTT[:, blk * 128:(blk + 1) * 128],
                        rhs=er_all[:, b * H:(b + 1) * H],
                        start=False, stop=True,
                    )
                    # single PSUM drain per block; everything else is batched
                    nc.scalar.activation(
                        out=Gsb[:, blk, :], in_=psG[:],
                        func=mybir.ActivationFunctionType.Copy)

                # batched scores for the whole chunk (z = el+er already in Gsb)
                zt3 = zt[:].rearrange("p (b h) -> p b h", h=H)
                es = np_.tile([128, nb * H], f32, tag="es")
                nc.vector.tensor_scalar_mul(
                    es[:].rearrange("p (b h) -> p b h", h=H),
                    Gsb[:, :, HC:PAY], NEG)
                nc.vector.tensor_tensor(
                    out=zt3, in0=Gsb[:, :, HC:PAY],
                    in1=es[:].rearrange("p (b h) -> p b h", h=H),
                    op=mybir.AluOpType.max)
                nc.scalar.activation(
                    out=w_t[:], in_=zt3,
                    func=mybir.ActivationFunctionType.Exp)
                # V rows, batched at 2x bf16 DVE rate
                V4 = V[:, :, 0:HC].rearrange("p b (h c) -> p b h c", c=C)
                G4 = Gsb[:, :, 0:HC].rearrange("p b (h c) -> p b h c", c=C)
                w4 = w_t[:].to_broadcast([128, nb, H, C])
                nc.vector.tensor_tensor(out=V4, in0=G4, in1=w4,
                                        op=mybir.AluOpType.mult)
                nc.scalar.activation(
                    out=V[:, :, HC:PAY], in_=w_t[:],
                    func=mybir.ActivationFunctionType.Copy)

                # scatter + normalize per bucket
                V2 = V[:].rearrange("p b y -> p (b y)")
                for bi, b in enumerate(ch["buckets"]):
                    ps = ps2p.tile([128, PAY], f32, tag="acc")
                    nbb = plan.caps[b]
                    for j in range(nbb):
                        blk = ch["boff"][bi] + j
                        nc.tensor.matmul(
                            out=ps[:],
                            lhsT=OT[:, blk * 128:(blk + 1) * 128],
                            rhs=V2[:, blk * PAY:(blk + 1) * PAY],
                            start=(j == 0), stop=(j == nbb - 1),
                        )
                    den = np_.tile([128, H], f32, tag="den")
                    nc.vector.tensor_scalar_add(den[:], ps[:, HC:PAY], EPS)
                    rec = np_.tile([128, H], f32, tag="rec")
                    nc.vector.reciprocal(rec[:], den[:])
                    ot = np_.tile([128, HC], f32, tag="ot")
                    ot3 = ot[:].rearrange("p (h c) -> p h c", c=C)
                    n3 = ps[:, 0:HC].rearrange("p (h c) -> p h c", c=C)
                    r3 = rec[:].to_broadcast([128, H, C])
                    nc.vector.tensor_tensor(out=ot3, in0=n3, in1=r3,
                                            op=mybir.AluOpType.mult)
                    rows = min(128, NPC - b * 128)
                    nc.sync.dma_start(
                        out=out_ext[b * 128:b * 128 + rows, :],
                        in_=ot[:rows, :])

    _split_excess_waits(nc)
    return nc


def kernel(**inputs):
    x = np.asarray(inputs["x"], np.float32)
    edge_index = np.asarray(inputs["edge_index"])
    W = np.asarray(inputs["W"], np.float32)
    a_left = np.asarray(inputs["a_left"], np.float32)
    a_right = np.asarray(inputs["a_right"], np.float32)

    plan, wtbW, wtbR, xgT, OTT, OTH, xT = _host_prep(
        x, edge_index, W, a_left, a_right)
    nc = _build_program(plan)

    in_maps = []
    for c in range(NC):
        in_maps.append({
            "xg0": np.ascontiguousarray(xgT[c, 0:128]),
            "xg1": np.ascontiguousarray(xgT[c, 128:256]),
            "ott": np.ascontiguousarray(OTT[c]),
            "xT": np.ascontiguousarray(xT[c]),
            "wtbW": wtbW,
            "wtbR": wtbR,
            "oth": np.ascontiguousarray(OTH[c]),
        })

    res = run_bass_kernel_spmd(nc, in_maps, core_ids=list(range(NC)))
    out = np.concatenate([np.asarray(res.results[c]["out"]) for c in range(NC)], axis=0)
    return out.astype(np.float32)
